# revision 28
# baseline (speedup 1.0000x reference)
"""Binarized linear kernel for Trainium2, 8 NeuronCores.

Computes out = sign(x) @ sign(W)^T * alpha + bias  for
x [4, 2048, 4096] f32, W [4096, 4096] f32, bias/alpha [4096] f32.

Sharding: R token-shards x C dout-shards = 8 cores (no collectives;
every core owns a disjoint output block).  The host pre-signs BOTH
operands to fp8e4 {-1,0,+1} (exact) and pre-tiles them so every DMA is
fully contiguous: x as [TT, 128, KT, 128] (partition dim = din-tile),
W as [KP, 128, 2, dout] k-pair blocks.  alpha/bias ship as an exact
3-way bf16 split and are broadcast across the 128 partitions with a
tiny K=3 matmul against ones (which doubles as PE warm-up).

On device the kernel is pure fp8 DoubleRow matmul (K=256 per pass,
fp32 PSUM, exact) + the f32 alpha/bias epilogue on vector.  No
activation-engine work at all, which frees the scalar engine to act
as the second HWDGE DMA ring (TRN2 has two: sync + scalar).

Schedule: the first rounds are HBM-delivery-bound (all 8 cores fetch
their first MBs simultaneously), so the first four token tiles run
k-pair-interleaved -- each W k-pair feeds 4 tiles' matmuls, keeping
PE consumption per byte of W delivered at 4x a single tile's.  DMAs
are issued in need-order across three queues (sync, scalar=HWDGE
ring 2, gpsimd=SWDGE): sync leads with W kp0 (it gates the first
real matmul; nothing posts ahead of it) then x0 stream + even W
k-pairs; scalar carries x0-head, ab3, odd W k-pairs + x1 stream;
gpsimd carries the x1/x2/x3 heads + x2/x3 streams.  20 tiny ones x
ones matmuls bridge the PE from engine-boot (~7 us) to first-data
(~10 us); the HAM clock gate only reaches 2.4 GHz after ~3.4 us of
the dense real stream, so a short cold segment at 427 ns/matmul is
unavoidable before the warm 216 ns/matmul steady state (the fp8
DoubleRow N=512 issue-rate roofline; the warm phase measures ~99%
of fp8 peak).  Steady tiles: x in on sync, out stores on scalar.
The final two token tiles run bank/quarter-split with a separate
PSUM tile per chunk (a chunk's start=True matmul WAR-depends on the
whole previous tile), stores alternating the two HWDGE rings, so
the last epilogue + store trail ~2.3 us behind the last matmul.
The result is bit-exact vs the fp32 reference.
"""

from contextlib import ExitStack

import numpy as np

import concourse.mybir as mybir
import concourse.tile as tile
from concourse import bacc
from concourse.bass import ts

B, S, DIN, DOUT_FULL = 4, 2048, 4096, 4096
NTOK = B * S

# sharding grid: R token shards x C dout shards
R, C = 2, 4
TOK = NTOK // R
DOUT = DOUT_FULL // C

# PE warm-up bridge length (tiny ones x ones matmuls)
NWARM = 20

# stash of the last BassKernelResults (for test.py to read profile info)
LAST_RESULTS = None


def build_nc(din=DIN, tok=TOK, dout=DOUT):
    """One NeuronCore program: out[tok, dout] = xb.T @ w8 * alpha + bias.

    Inputs (per core):
      xt  [tok//128, 128, din//128, 128] fp8e4 : sign(x) shard, transposed
          + tiled (din-partition dim second) so each token-tile and each
          4-ktile chunk is contiguous
      wt  [din//256, 128, 2, dout] fp8e4 : sign(W) shard, transposed +
          tiled per k-pair so each k-pair DMA is contiguous
      ab3 [3, 2*dout] bf16 : exact 3-way bf16 split of concat(alpha, bias)
    Output: out [tok, dout] f32
    """
    f32 = mybir.dt.float32
    bf16 = mybir.dt.bfloat16
    fp8 = mybir.dt.float8e4
    nc = bacc.Bacc("TRN2", target_bir_lowering=False)

    P = 128
    KT = din // P          # contraction tiles (128 wide)
    KP = KT // 2           # k-pairs (256 wide, DoubleRow)
    TT = tok // P          # token tiles
    NB = dout // 512       # psum banks per output row-tile

    xt = nc.declare_dram_parameter("xt", [TT, P, KT, P], fp8, isOutput=False)
    wt = nc.declare_dram_parameter("wt", [KP, P, 2, dout], fp8, isOutput=False)
    ab3 = nc.declare_dram_parameter("ab3", [3, 2 * dout], bf16, isOutput=False)
    out = nc.declare_dram_parameter("out", [tok, dout], f32, isOutput=True)

    with ExitStack() as ctx:
        tc = ctx.enter_context(tile.TileContext(nc))
        consts = ctx.enter_context(tc.tile_pool(name="consts", bufs=1))
        wpool = ctx.enter_context(tc.tile_pool(name="wpool", bufs=1))
        xpool = ctx.enter_context(tc.tile_pool(name="xpool", bufs=8))
        opool = ctx.enter_context(tc.tile_pool(name="opool", bufs=4))
        pspool = ctx.enter_context(tc.tile_pool(name="psum", bufs=4, space="PSUM"))

        wsb = wpool.tile([P, KT, dout], fp8)

        def w_dma(eng, kp):
            eng.dma_start(out=wsb[:, 2 * kp:2 * kp + 2, :], in_=wt[kp])

        # x tiles land pre-signed; leading tiles stream in 4-ktile chunks
        xbs = {}

        def x_alloc(t):
            xbs[t] = xpool.tile([P, KT, P], fp8, tag="xb", name=f"xb{t}")

        def x_dma(eng, t, k0, k1):
            eng.dma_start(out=xbs[t][:, k0:k1, :], in_=xt[t][:, k0:k1, :])

        for t in range(4):
            x_alloc(t)

        # --- DMA issue order (need-order, three queues) ----------------
        # Round r of the interleaved leading phase consumes W k-pair r
        # (256 KB) plus 2 ktiles of each of x0-x3 (4 x 32 KB).  Each
        # queue's FIFO carries its share in exactly need order (the
        # queue itself paces them).  W0 gates the first real matmul, so
        # it is the very first descriptor posted anywhere (sync ring,
        # no post ahead of it); the x heads ride the other two queues.
        #   sync   : W0, then x0 chunks + even W k-pairs
        #   scalar : x0 head, ab3, W1, then x1 chunks + odd W k-pairs
        #   gpsimd : x1/x2/x3 heads + x2/x3 chunks
        abst = consts.tile([3, 2 * dout], bf16)
        # W kp0 is split across BOTH HWDGE rings so its two dout-halves
        # transfer in parallel: the first matmul needs only h0 (sync's
        # first descriptor), h1 follows on scalar one cold-matmul later.
        # W parity then swaps (sync: odd k-pairs, scalar: even) so W2 --
        # which gates round 2 -- rides scalar's front instead of sitting
        # behind W0+x0 on sync.
        nc.sync.dma_start(out=wsb[:, 0:2, 0:512], in_=wt[0][:, :, 0:512])
        nc.scalar.dma_start(out=abst, in_=ab3[:])
        x_dma(nc.scalar, 0, 0, 4)
        nc.scalar.dma_start(out=wsb[:, 0:2, 512:], in_=wt[0][:, :, 512:])
        w_dma(nc.sync, 1)
        w_dma(nc.scalar, 2)
        ones = consts.tile([3, P], bf16)
        nc.gpsimd.memset(ones, 1.0)
        x_dma(nc.gpsimd, 1, 0, 4)
        x_dma(nc.gpsimd, 2, 0, 4)
        x_dma(nc.gpsimd, 3, 0, 4)
        for j in range(1, 8):
            x_dma(nc.sync, 0, 4 * j, 4 * j + 4)
            if 2 * j + 1 < KP:
                w_dma(nc.sync, 2 * j + 1)
            x_dma(nc.scalar, 1, 4 * j, 4 * j + 4)
            if 2 * j + 2 < KP:
                w_dma(nc.scalar, 2 * j + 2)
            x_dma(nc.gpsimd, 2, 4 * j, 4 * j + 4)
            x_dma(nc.gpsimd, 3, 4 * j, 4 * j + 4)

        # --- alpha/bias broadcast via PE (also PE warm-up) ------------
        # NWARM tiny ones x ones matmuls first: they only depend on the
        # gpsimd memset, so the PE is busy from ~7.4 us; the bridge is
        # sized so the HAM clock gate reaches 2.4 GHz (3.4 us of busy)
        # and ~2 rounds of x/W are buffered before the first real
        # matmul issues -- the data-dependent stream then runs warm
        # with no delivery stalls
        ps_a = pspool.tile([P, dout], f32, tag="ps", name="ps_alpha")
        for _ in range(NWARM):
            nc.tensor.matmul(ps_a[:, 0:P], lhsT=ones, rhs=ones,
                             start=True, stop=True)
        for h in range(NB):
            nc.tensor.matmul(ps_a[:, ts(h, 512)], lhsT=ones,
                             rhs=abst[:, ts(h, 512)], start=True, stop=True)
        ps_b = pspool.tile([P, dout], f32, tag="ps", name="ps_bias")
        for h in range(NB):
            nc.tensor.matmul(ps_b[:, ts(h, 512)], lhsT=ones,
                             rhs=abst[:, dout + h * 512:dout + (h + 1) * 512],
                             start=True, stop=True)
        alphaB = consts.tile([P, dout], f32)
        biasB = consts.tile([P, dout], f32)
        nc.vector.tensor_scalar_add(alphaB, ps_a, 0.0)
        nc.vector.tensor_scalar_add(biasB, ps_b, 0.0)

        def epilogue(psd, t):
            osb = opool.tile([P, dout], f32, tag="osb", name=f"osb{t}")
            nc.vector.tensor_mul(osb, psd, alphaB)
            nc.vector.tensor_add(osb, osb, biasB)
            nc.scalar.dma_start(out=out[ts(t, P), :], in_=osb)

        def mm_pair(psd, t, kp):
            for b2 in range(NB):
                nc.tensor.matmul(
                    psd[:, ts(b2, 512)],
                    lhsT=xbs[t][:, 2 * kp:2 * kp + 2, :],
                    rhs=wsb[:, 2 * kp:2 * kp + 2, ts(b2, 512)],
                    start=(kp == 0),
                    stop=(kp == KP - 1),
                    perf_mode=mybir.MatmulPerfMode.DoubleRow,
                )

        # --- leading tiles 0-3: k-pair-interleaved rounds -------------
        # near-zero lags: all four tiles consume W k-pair r in round r,
        # so PE demand per unit W delivered is 4x a single tile's, which
        # keeps consumption below the early DMA delivery rate; staggered
        # by one round so the four epilogues (and PSUM frees) stagger too
        LAGS = (0, 1, 2, 3)
        pstile = {}
        for t in range(4):
            pstile[t] = pspool.tile([P, dout], f32, tag="ps", name=f"ps_t{t}")
        for r in range(KP + LAGS[-1]):
            for t in range(4):
                kp = r - LAGS[t]
                if 0 <= kp < KP:
                    mm_pair(pstile[t], t, kp)
                    if kp == KP - 1:
                        epilogue(pstile[t], t)

        # --- steady-state tiles ---------------------------------------
        for t in range(4, TT):
            x_alloc(t)
            x_dma(nc.sync, t, 0, 32)
            if t < TT - 2:
                psd = pspool.tile([P, dout], f32, tag="ps", name=f"ps_t{t}")
                for kp in range(KP):
                    mm_pair(psd, t, kp)
                epilogue(psd, t)
            else:
                # tail tiles: chunked, each chunk with its OWN psum tile
                # (a chunk's start=True matmul WAR-depends on the whole
                # previous tile -> separate tiles avoid serializing
                # against the prior chunk's epilogue read).  The very
                # last chunk is narrow so the final epilogue + store
                # trail as little as possible behind the last matmul;
                # stores alternate the two HWDGE rings so posting and
                # completion overlap.
                widths = [512, 512] if t == TT - 2 else [320, 320, 256, 128]
                c0 = 0
                for q, w in enumerate(widths):
                    psq = pspool.tile([P, w], f32, tag="ps",
                                      name=f"ps_t{t}_q{q}")
                    for kp in range(KP):
                        nc.tensor.matmul(
                            psq,
                            lhsT=xbs[t][:, 2 * kp:2 * kp + 2, :],
                            rhs=wsb[:, 2 * kp:2 * kp + 2, c0:c0 + w],
                            start=(kp == 0),
                            stop=(kp == KP - 1),
                            perf_mode=mybir.MatmulPerfMode.DoubleRow,
                        )
                    oq = opool.tile([P, w], f32, tag="osb",
                                    name=f"osb_q{t}_{q}")
                    nc.vector.tensor_mul(oq, psq, alphaB[:, c0:c0 + w])
                    nc.vector.tensor_add(oq, oq, biasB[:, c0:c0 + w])
                    oeng = nc.sync if q % 2 == 0 else nc.scalar
                    oeng.dma_start(out=out[ts(t, P), c0:c0 + w], in_=oq)
                    c0 += w
    nc.finalize()
    return nc


def _split3_bf16(v):
    """Exact 3-way bf16 split: v == hi + mid + lo (f32 sum), elementwise."""
    import ml_dtypes

    bf16 = ml_dtypes.bfloat16
    v = np.asarray(v, dtype=np.float32)
    hi = v.astype(bf16)
    r = v - hi.astype(np.float32)
    mid = r.astype(bf16)
    r2 = r - mid.astype(np.float32)
    lo = r2.astype(bf16)
    assert (hi.astype(np.float32) + mid.astype(np.float32)
            + lo.astype(np.float32) == v).all(), "bf16 3-split not exact"
    return np.stack([hi, mid, lo])


def _shard_inputs(x, weight, bias, alpha):
    import ml_dtypes

    fp8 = ml_dtypes.float8_e4m3
    P = 128
    KT = DIN // P
    KP = KT // 2
    TT = TOK // P

    x2 = np.asarray(x, dtype=np.float32).reshape(NTOK, DIN)
    w = np.asarray(weight, dtype=np.float32)
    bias = np.asarray(bias, dtype=np.float32).reshape(-1)
    alpha_f = np.asarray(alpha, dtype=np.float32).reshape(-1)

    xTs = []
    for r in range(R):
        xT = np.sign(x2[r * TOK:(r + 1) * TOK, :].T).astype(fp8)  # [DIN, TOK]
        # -> [TT, 128(p), KT, 128(t)]: each [p, kt, t] token-tile contiguous
        xt_tiled = np.ascontiguousarray(
            xT.reshape(KT, P, TT, P).transpose(2, 1, 0, 3))
        xTs.append(xt_tiled)

    wT8 = np.sign(w.T).astype(fp8)  # [DIN, DOUT_FULL], values in {-1, 0, +1}
    wts, abs3 = [], []
    for c in range(C):
        blk = wT8[:, c * DOUT:(c + 1) * DOUT]  # [DIN, DOUT]
        # -> [KP, 128(p), 2, DOUT]: each k-pair contiguous, partition p
        # holds din rows 256*kp + p and 256*kp + 128 + p
        wt_tiled = np.ascontiguousarray(
            blk.reshape(KP, 2, P, DOUT).transpose(0, 2, 1, 3))
        wts.append(wt_tiled)
        ab = np.concatenate([alpha_f[c * DOUT:(c + 1) * DOUT],
                             bias[c * DOUT:(c + 1) * DOUT]])
        abs3.append(_split3_bf16(ab))

    in_maps = []
    for i in range(8):
        r, c = divmod(i, C)
        in_maps.append({"xt": xTs[r], "wt": wts[c], "ab3": abs3[c]})
    return in_maps


def kernel(x, weight, bias, alpha, _trace=False, _trace_cores=None):
    global LAST_RESULTS
    from concourse.bass_utils import run_bass_kernel_spmd

    in_maps = _shard_inputs(x, weight, bias, alpha)
    nc = build_nc()
    kwargs = {}
    if _trace:
        kwargs = dict(trace=True, trace_cores=_trace_cores or [0])
    res = run_bass_kernel_spmd(nc, in_maps, core_ids=list(range(8)), **kwargs)
    LAST_RESULTS = res

    out = np.empty((NTOK, DOUT_FULL), dtype=np.float32)
    for i in range(8):
        r, c = divmod(i, C)
        out[r * TOK:(r + 1) * TOK, c * DOUT:(c + 1) * DOUT] = res.results[i]["out"]
    return out.reshape(B, S, DOUT_FULL)


# revision 29
# speedup vs baseline: 1.0079x; 1.0079x over previous
"""Binarized linear kernel for Trainium2, 8 NeuronCores.

Computes out = sign(x) @ sign(W)^T * alpha + bias  for
x [4, 2048, 4096] f32, W [4096, 4096] f32, bias/alpha [4096] f32.

Sharding: R token-shards x C dout-shards = 8 cores (no collectives;
every core owns a disjoint output block).  The host pre-signs BOTH
operands to fp8e4 {-1,0,+1} (exact) and pre-tiles them so every DMA is
fully contiguous: x as [TT, 128, KT, 128] (partition dim = din-tile),
W as [KP, 128, 2, dout] k-pair blocks.  alpha/bias ship as an exact
3-way bf16 split and are broadcast across the 128 partitions with a
tiny K=3 matmul against ones (which doubles as PE warm-up).

On device the kernel is pure fp8 DoubleRow matmul (K=256 per pass,
fp32 PSUM, exact) + the f32 alpha/bias epilogue on vector.  No
activation-engine work at all, which frees the scalar engine to act
as the second HWDGE DMA ring (TRN2 has two: sync + scalar).

Schedule: the first rounds are HBM-delivery-bound (all 8 cores fetch
their first MBs simultaneously), so the first four token tiles run
k-pair-interleaved -- each W k-pair feeds 4 tiles' matmuls, keeping
PE consumption per byte of W delivered at 4x a single tile's.  DMAs
are issued in need-order across three queues (sync, scalar=HWDGE
ring 2, gpsimd=SWDGE): sync leads with W kp0 (it gates the first
real matmul; nothing posts ahead of it) then x0 stream + even W
k-pairs; scalar carries x0-head, ab3, odd W k-pairs + x1 stream;
gpsimd carries the x1/x2/x3 heads + x2/x3 streams.  20 tiny ones x
ones matmuls bridge the PE from engine-boot (~7 us) to first-data
(~10 us); the HAM clock gate only reaches 2.4 GHz after ~3.4 us of
the dense real stream, so a short cold segment at 427 ns/matmul is
unavoidable before the warm 216 ns/matmul steady state (the fp8
DoubleRow N=512 issue-rate roofline; the warm phase measures ~99%
of fp8 peak).  Steady tiles: x in on sync, out stores on scalar.
The final two token tiles run bank/quarter-split with a separate
PSUM tile per chunk (a chunk's start=True matmul WAR-depends on the
whole previous tile), stores alternating the two HWDGE rings, so
the last epilogue + store trail ~2.3 us behind the last matmul.
The result is bit-exact vs the fp32 reference.
"""

from contextlib import ExitStack

import numpy as np

import concourse.mybir as mybir
import concourse.tile as tile
from concourse import bacc
from concourse.bass import ts

B, S, DIN, DOUT_FULL = 4, 2048, 4096, 4096
NTOK = B * S

# sharding grid: R token shards x C dout shards
R, C = 2, 4
TOK = NTOK // R
DOUT = DOUT_FULL // C

# PE warm-up bridge length (tiny ones x ones matmuls)
NWARM = 20

# stash of the last BassKernelResults (for test.py to read profile info)
LAST_RESULTS = None


def build_nc(din=DIN, tok=TOK, dout=DOUT):
    """One NeuronCore program: out[tok, dout] = xb.T @ w8 * alpha + bias.

    Inputs (per core):
      xt  [tok//128, 128, din//128, 128] fp8e4 : sign(x) shard, transposed
          + tiled (din-partition dim second) so each token-tile and each
          4-ktile chunk is contiguous
      wt  [din//256, 128, 2, dout] fp8e4 : sign(W) shard, transposed +
          tiled per k-pair so each k-pair DMA is contiguous
      ab3 [3, 2*dout] bf16 : exact 3-way bf16 split of concat(alpha, bias)
    Output: out [tok, dout] f32
    """
    f32 = mybir.dt.float32
    bf16 = mybir.dt.bfloat16
    fp8 = mybir.dt.float8e4
    nc = bacc.Bacc("TRN2", target_bir_lowering=False)

    P = 128
    KT = din // P          # contraction tiles (128 wide)
    KP = KT // 2           # k-pairs (256 wide, DoubleRow)
    TT = tok // P          # token tiles
    NB = dout // 512       # psum banks per output row-tile

    xt = nc.declare_dram_parameter("xt", [TT, P, KT, P], fp8, isOutput=False)
    wt = nc.declare_dram_parameter("wt", [KP, P, 2, dout], fp8, isOutput=False)
    ab3 = nc.declare_dram_parameter("ab3", [3, 2 * dout], bf16, isOutput=False)
    out = nc.declare_dram_parameter("out", [tok, dout], f32, isOutput=True)

    with ExitStack() as ctx:
        tc = ctx.enter_context(tile.TileContext(nc))
        consts = ctx.enter_context(tc.tile_pool(name="consts", bufs=1))
        wpool = ctx.enter_context(tc.tile_pool(name="wpool", bufs=1))
        xpool = ctx.enter_context(tc.tile_pool(name="xpool", bufs=8))
        opool = ctx.enter_context(tc.tile_pool(name="opool", bufs=4))
        pspool = ctx.enter_context(tc.tile_pool(name="psum", bufs=4, space="PSUM"))

        wsb = wpool.tile([P, KT, dout], fp8)

        def w_dma(eng, kp):
            eng.dma_start(out=wsb[:, 2 * kp:2 * kp + 2, :], in_=wt[kp])

        # x tiles land pre-signed; leading tiles stream in 4-ktile chunks
        xbs = {}

        def x_alloc(t):
            xbs[t] = xpool.tile([P, KT, P], fp8, tag="xb", name=f"xb{t}")

        def x_dma(eng, t, k0, k1):
            eng.dma_start(out=xbs[t][:, k0:k1, :], in_=xt[t][:, k0:k1, :])

        for t in range(4):
            x_alloc(t)

        # --- DMA issue order (need-order, three queues) ----------------
        # Round r of the interleaved leading phase consumes W k-pair r
        # (256 KB) plus 2 ktiles of each of x0-x3 (4 x 32 KB).  Each
        # queue's FIFO carries its share in exactly need order (the
        # queue itself paces them).  W0 gates the first real matmul, so
        # it is the very first descriptor posted anywhere (sync ring,
        # no post ahead of it); the x heads ride the other two queues.
        #   sync   : W0, then x0 chunks + even W k-pairs
        #   scalar : x0 head, ab3, W1, then x1 chunks + odd W k-pairs
        #   gpsimd : x1/x2/x3 heads + x2/x3 chunks
        abst = consts.tile([3, 2 * dout], bf16)
        # W kp0 is split across BOTH HWDGE rings so its two dout-halves
        # transfer in parallel: the first matmul needs only h0 (sync's
        # first descriptor), h1 follows on scalar one cold-matmul later.
        # W parity then swaps (sync: odd k-pairs, scalar: even) so W2 --
        # which gates round 2 -- rides scalar's front instead of sitting
        # behind W0+x0 on sync.
        nc.sync.dma_start(out=wsb[:, 0:2, 0:512], in_=wt[0][:, :, 0:512])
        x_dma(nc.scalar, 0, 0, 4)
        nc.scalar.dma_start(out=abst, in_=ab3[:])
        nc.scalar.dma_start(out=wsb[:, 0:2, 512:], in_=wt[0][:, :, 512:])
        w_dma(nc.sync, 1)
        w_dma(nc.scalar, 2)
        ones = consts.tile([3, P], bf16)
        nc.gpsimd.memset(ones, 1.0)
        x_dma(nc.gpsimd, 1, 0, 4)
        x_dma(nc.gpsimd, 2, 0, 4)
        x_dma(nc.gpsimd, 3, 0, 4)
        for j in range(1, 8):
            x_dma(nc.sync, 0, 4 * j, 4 * j + 4)
            if 2 * j + 1 < KP:
                w_dma(nc.sync, 2 * j + 1)
            x_dma(nc.scalar, 1, 4 * j, 4 * j + 4)
            if 2 * j + 2 < KP:
                w_dma(nc.scalar, 2 * j + 2)
            x_dma(nc.gpsimd, 2, 4 * j, 4 * j + 4)
            x_dma(nc.gpsimd, 3, 4 * j, 4 * j + 4)

        # --- alpha/bias broadcast via PE (also PE warm-up) ------------
        # NWARM tiny ones x ones matmuls first: they only depend on the
        # gpsimd memset, so the PE is busy from ~7.4 us; the bridge is
        # sized so the HAM clock gate reaches 2.4 GHz (3.4 us of busy)
        # and ~2 rounds of x/W are buffered before the first real
        # matmul issues -- the data-dependent stream then runs warm
        # with no delivery stalls
        ps_a = pspool.tile([P, dout], f32, tag="ps", name="ps_alpha")
        for _ in range(NWARM):
            nc.tensor.matmul(ps_a[:, 0:P], lhsT=ones, rhs=ones,
                             start=True, stop=True)
        for h in range(NB):
            nc.tensor.matmul(ps_a[:, ts(h, 512)], lhsT=ones,
                             rhs=abst[:, ts(h, 512)], start=True, stop=True)
        ps_b = pspool.tile([P, dout], f32, tag="ps", name="ps_bias")
        for h in range(NB):
            nc.tensor.matmul(ps_b[:, ts(h, 512)], lhsT=ones,
                             rhs=abst[:, dout + h * 512:dout + (h + 1) * 512],
                             start=True, stop=True)
        alphaB = consts.tile([P, dout], f32)
        biasB = consts.tile([P, dout], f32)
        nc.vector.tensor_scalar_add(alphaB, ps_a, 0.0)
        nc.vector.tensor_scalar_add(biasB, ps_b, 0.0)

        def epilogue(psd, t):
            osb = opool.tile([P, dout], f32, tag="osb", name=f"osb{t}")
            nc.vector.tensor_mul(osb, psd, alphaB)
            nc.vector.tensor_add(osb, osb, biasB)
            nc.scalar.dma_start(out=out[ts(t, P), :], in_=osb)

        def mm_pair(psd, t, kp):
            for b2 in range(NB):
                nc.tensor.matmul(
                    psd[:, ts(b2, 512)],
                    lhsT=xbs[t][:, 2 * kp:2 * kp + 2, :],
                    rhs=wsb[:, 2 * kp:2 * kp + 2, ts(b2, 512)],
                    start=(kp == 0),
                    stop=(kp == KP - 1),
                    perf_mode=mybir.MatmulPerfMode.DoubleRow,
                )

        # --- leading tiles 0-3: k-pair-interleaved rounds -------------
        # near-zero lags: all four tiles consume W k-pair r in round r,
        # so PE demand per unit W delivered is 4x a single tile's, which
        # keeps consumption below the early DMA delivery rate; staggered
        # by one round so the four epilogues (and PSUM frees) stagger too
        LAGS = (0, 1, 2, 3)
        pstile = {}
        for t in range(4):
            pstile[t] = pspool.tile([P, dout], f32, tag="ps", name=f"ps_t{t}")
        for r in range(KP + LAGS[-1]):
            for t in range(4):
                kp = r - LAGS[t]
                if 0 <= kp < KP:
                    mm_pair(pstile[t], t, kp)
                    if kp == KP - 1:
                        epilogue(pstile[t], t)

        # --- steady-state tiles ---------------------------------------
        for t in range(4, TT):
            x_alloc(t)
            x_dma(nc.sync, t, 0, 32)
            if t < TT - 2:
                psd = pspool.tile([P, dout], f32, tag="ps", name=f"ps_t{t}")
                for kp in range(KP):
                    mm_pair(psd, t, kp)
                epilogue(psd, t)
            else:
                # tail tiles: chunked, each chunk with its OWN psum tile
                # (a chunk's start=True matmul WAR-depends on the whole
                # previous tile -> separate tiles avoid serializing
                # against the prior chunk's epilogue read).  The very
                # last chunk is narrow so the final epilogue + store
                # trail as little as possible behind the last matmul;
                # stores alternate the two HWDGE rings so posting and
                # completion overlap.
                widths = [512, 512] if t == TT - 2 else [320, 320, 256, 128]
                c0 = 0
                for q, w in enumerate(widths):
                    psq = pspool.tile([P, w], f32, tag="ps",
                                      name=f"ps_t{t}_q{q}")
                    for kp in range(KP):
                        nc.tensor.matmul(
                            psq,
                            lhsT=xbs[t][:, 2 * kp:2 * kp + 2, :],
                            rhs=wsb[:, 2 * kp:2 * kp + 2, c0:c0 + w],
                            start=(kp == 0),
                            stop=(kp == KP - 1),
                            perf_mode=mybir.MatmulPerfMode.DoubleRow,
                        )
                    oq = opool.tile([P, w], f32, tag="osb",
                                    name=f"osb_q{t}_{q}")
                    nc.vector.tensor_mul(oq, psq, alphaB[:, c0:c0 + w])
                    nc.vector.tensor_add(oq, oq, biasB[:, c0:c0 + w])
                    oeng = nc.sync if q % 2 == 0 else nc.scalar
                    oeng.dma_start(out=out[ts(t, P), c0:c0 + w], in_=oq)
                    c0 += w
    nc.finalize()
    return nc


def _split3_bf16(v):
    """Exact 3-way bf16 split: v == hi + mid + lo (f32 sum), elementwise."""
    import ml_dtypes

    bf16 = ml_dtypes.bfloat16
    v = np.asarray(v, dtype=np.float32)
    hi = v.astype(bf16)
    r = v - hi.astype(np.float32)
    mid = r.astype(bf16)
    r2 = r - mid.astype(np.float32)
    lo = r2.astype(bf16)
    assert (hi.astype(np.float32) + mid.astype(np.float32)
            + lo.astype(np.float32) == v).all(), "bf16 3-split not exact"
    return np.stack([hi, mid, lo])


def _shard_inputs(x, weight, bias, alpha):
    import ml_dtypes

    fp8 = ml_dtypes.float8_e4m3
    P = 128
    KT = DIN // P
    KP = KT // 2
    TT = TOK // P

    x2 = np.asarray(x, dtype=np.float32).reshape(NTOK, DIN)
    w = np.asarray(weight, dtype=np.float32)
    bias = np.asarray(bias, dtype=np.float32).reshape(-1)
    alpha_f = np.asarray(alpha, dtype=np.float32).reshape(-1)

    xTs = []
    for r in range(R):
        xT = np.sign(x2[r * TOK:(r + 1) * TOK, :].T).astype(fp8)  # [DIN, TOK]
        # -> [TT, 128(p), KT, 128(t)]: each [p, kt, t] token-tile contiguous
        xt_tiled = np.ascontiguousarray(
            xT.reshape(KT, P, TT, P).transpose(2, 1, 0, 3))
        xTs.append(xt_tiled)

    wT8 = np.sign(w.T).astype(fp8)  # [DIN, DOUT_FULL], values in {-1, 0, +1}
    wts, abs3 = [], []
    for c in range(C):
        blk = wT8[:, c * DOUT:(c + 1) * DOUT]  # [DIN, DOUT]
        # -> [KP, 128(p), 2, DOUT]: each k-pair contiguous, partition p
        # holds din rows 256*kp + p and 256*kp + 128 + p
        wt_tiled = np.ascontiguousarray(
            blk.reshape(KP, 2, P, DOUT).transpose(0, 2, 1, 3))
        wts.append(wt_tiled)
        ab = np.concatenate([alpha_f[c * DOUT:(c + 1) * DOUT],
                             bias[c * DOUT:(c + 1) * DOUT]])
        abs3.append(_split3_bf16(ab))

    in_maps = []
    for i in range(8):
        r, c = divmod(i, C)
        in_maps.append({"xt": xTs[r], "wt": wts[c], "ab3": abs3[c]})
    return in_maps


def kernel(x, weight, bias, alpha, _trace=False, _trace_cores=None):
    global LAST_RESULTS
    from concourse.bass_utils import run_bass_kernel_spmd

    in_maps = _shard_inputs(x, weight, bias, alpha)
    nc = build_nc()
    kwargs = {}
    if _trace:
        kwargs = dict(trace=True, trace_cores=_trace_cores or [0])
    res = run_bass_kernel_spmd(nc, in_maps, core_ids=list(range(8)), **kwargs)
    LAST_RESULTS = res

    out = np.empty((NTOK, DOUT_FULL), dtype=np.float32)
    for i in range(8):
        r, c = divmod(i, C)
        out[r * TOK:(r + 1) * TOK, c * DOUT:(c + 1) * DOUT] = res.results[i]["out"]
    return out.reshape(B, S, DOUT_FULL)


# revision 30
# speedup vs baseline: 1.0103x; 1.0024x over previous
"""Binarized linear kernel for Trainium2, 8 NeuronCores.

Computes out = sign(x) @ sign(W)^T * alpha + bias  for
x [4, 2048, 4096] f32, W [4096, 4096] f32, bias/alpha [4096] f32.

Sharding: R token-shards x C dout-shards = 8 cores (no collectives;
every core owns a disjoint output block).  The host pre-signs BOTH
operands to fp8e4 {-1,0,+1} (exact) and pre-tiles them so every DMA is
fully contiguous: x as [TT, 128, KT, 128] (partition dim = din-tile),
W as [KP, 128, 2, dout] k-pair blocks.  alpha/bias ship as an exact
3-way bf16 split and are broadcast across the 128 partitions with a
tiny K=3 matmul against ones (which doubles as PE warm-up).

On device the kernel is pure fp8 DoubleRow matmul (K=256 per pass,
fp32 PSUM, exact) + the f32 alpha/bias epilogue on vector.  No
activation-engine work at all, which frees the scalar engine to act
as the second HWDGE DMA ring (TRN2 has two: sync + scalar).

Schedule: the first rounds are HBM-delivery-bound (all 8 cores fetch
their first MBs simultaneously), so the first four token tiles run
k-pair-interleaved -- each W k-pair feeds 4 tiles' matmuls, keeping
PE consumption per byte of W delivered at 4x a single tile's.  DMAs
are issued in need-order across three queues (sync, scalar=HWDGE
ring 2, gpsimd=SWDGE): sync leads with W kp0 (it gates the first
real matmul; nothing posts ahead of it) then x0 stream + even W
k-pairs; scalar carries x0-head, ab3, odd W k-pairs + x1 stream;
gpsimd carries the x1/x2/x3 heads + x2/x3 streams.  20 tiny ones x
ones matmuls bridge the PE from engine-boot (~7 us) to first-data
(~10 us); the HAM clock gate only reaches 2.4 GHz after ~3.4 us of
the dense real stream, so a short cold segment at 427 ns/matmul is
unavoidable before the warm 216 ns/matmul steady state (the fp8
DoubleRow N=512 issue-rate roofline; the warm phase measures ~99%
of fp8 peak).  Steady tiles: x in on sync, out stores on scalar.
The final two token tiles run bank/quarter-split with a separate
PSUM tile per chunk (a chunk's start=True matmul WAR-depends on the
whole previous tile), stores alternating the two HWDGE rings, so
the last epilogue + store trail ~2.3 us behind the last matmul.
The result is bit-exact vs the fp32 reference.
"""

from contextlib import ExitStack

import numpy as np

import concourse.mybir as mybir
import concourse.tile as tile
from concourse import bacc
from concourse.bass import ts

B, S, DIN, DOUT_FULL = 4, 2048, 4096, 4096
NTOK = B * S

# sharding grid: R token shards x C dout shards
R, C = 2, 4
TOK = NTOK // R
DOUT = DOUT_FULL // C

# PE warm-up bridge length (tiny ones x ones matmuls)
NWARM = 20

# stash of the last BassKernelResults (for test.py to read profile info)
LAST_RESULTS = None


def build_nc(din=DIN, tok=TOK, dout=DOUT):
    """One NeuronCore program: out[tok, dout] = xb.T @ w8 * alpha + bias.

    Inputs (per core):
      xt  [tok//128, 128, din//128, 128] fp8e4 : sign(x) shard, transposed
          + tiled (din-partition dim second) so each token-tile and each
          4-ktile chunk is contiguous
      wt  [din//256, 128, 2, dout] fp8e4 : sign(W) shard, transposed +
          tiled per k-pair so each k-pair DMA is contiguous
      ab3 [3, 2*dout] bf16 : exact 3-way bf16 split of concat(alpha, bias)
    Output: out [tok, dout] f32
    """
    f32 = mybir.dt.float32
    bf16 = mybir.dt.bfloat16
    fp8 = mybir.dt.float8e4
    nc = bacc.Bacc("TRN2", target_bir_lowering=False)

    P = 128
    KT = din // P          # contraction tiles (128 wide)
    KP = KT // 2           # k-pairs (256 wide, DoubleRow)
    TT = tok // P          # token tiles
    NB = dout // 512       # psum banks per output row-tile

    xt = nc.declare_dram_parameter("xt", [TT, P, KT, P], fp8, isOutput=False)
    wt = nc.declare_dram_parameter("wt", [KP, P, 2, dout], fp8, isOutput=False)
    ab3 = nc.declare_dram_parameter("ab3", [3, 2 * dout], bf16, isOutput=False)
    out = nc.declare_dram_parameter("out", [tok, dout], f32, isOutput=True)

    with ExitStack() as ctx:
        tc = ctx.enter_context(tile.TileContext(nc))
        consts = ctx.enter_context(tc.tile_pool(name="consts", bufs=1))
        wpool = ctx.enter_context(tc.tile_pool(name="wpool", bufs=1))
        xpool = ctx.enter_context(tc.tile_pool(name="xpool", bufs=8))
        opool = ctx.enter_context(tc.tile_pool(name="opool", bufs=4))
        pspool = ctx.enter_context(tc.tile_pool(name="psum", bufs=4, space="PSUM"))

        wsb = wpool.tile([P, KT, dout], fp8)

        def w_dma(eng, kp):
            eng.dma_start(out=wsb[:, 2 * kp:2 * kp + 2, :], in_=wt[kp])

        # x tiles land pre-signed; leading tiles stream in 4-ktile chunks
        xbs = {}

        def x_alloc(t):
            xbs[t] = xpool.tile([P, KT, P], fp8, tag="xb", name=f"xb{t}")

        def x_dma(eng, t, k0, k1):
            eng.dma_start(out=xbs[t][:, k0:k1, :], in_=xt[t][:, k0:k1, :])

        for t in range(4):
            x_alloc(t)

        # --- DMA issue order (need-order, three queues) ----------------
        # Round r of the interleaved leading phase consumes W k-pair r
        # (256 KB) plus 2 ktiles of each of x0-x3 (4 x 32 KB).  Each
        # queue's FIFO carries its share in exactly need order (the
        # queue itself paces them).  W0 gates the first real matmul, so
        # it is the very first descriptor posted anywhere (sync ring,
        # no post ahead of it); the x heads ride the other two queues.
        #   sync   : W0, then x0 chunks + even W k-pairs
        #   scalar : x0 head, ab3, W1, then x1 chunks + odd W k-pairs
        #   gpsimd : x1/x2/x3 heads + x2/x3 chunks
        abst = consts.tile([3, 2 * dout], bf16)
        w_dma(nc.sync, 0)
        x_dma(nc.scalar, 0, 0, 4)
        nc.scalar.dma_start(out=abst, in_=ab3[:])
        w_dma(nc.scalar, 1)
        ones = consts.tile([3, P], bf16)
        nc.gpsimd.memset(ones, 1.0)
        x_dma(nc.gpsimd, 1, 0, 4)
        x_dma(nc.gpsimd, 2, 0, 4)
        x_dma(nc.gpsimd, 3, 0, 4)
        for j in range(1, 8):
            x_dma(nc.sync, 0, 4 * j, 4 * j + 4)
            if 2 * j < KP:
                w_dma(nc.sync, 2 * j)
            x_dma(nc.scalar, 1, 4 * j, 4 * j + 4)
            if 2 * j + 1 < KP:
                w_dma(nc.scalar, 2 * j + 1)
            x_dma(nc.gpsimd, 2, 4 * j, 4 * j + 4)
            x_dma(nc.gpsimd, 3, 4 * j, 4 * j + 4)

        # --- alpha/bias broadcast via PE (also PE warm-up) ------------
        # NWARM tiny ones x ones matmuls first: they only depend on the
        # gpsimd memset, so the PE is busy from ~7.4 us; the bridge is
        # sized so the HAM clock gate reaches 2.4 GHz (3.4 us of busy)
        # and ~2 rounds of x/W are buffered before the first real
        # matmul issues -- the data-dependent stream then runs warm
        # with no delivery stalls
        ps_a = pspool.tile([P, dout], f32, tag="ps", name="ps_alpha")
        for _ in range(NWARM):
            nc.tensor.matmul(ps_a[:, 0:P], lhsT=ones, rhs=ones,
                             start=True, stop=True)
        for h in range(NB):
            nc.tensor.matmul(ps_a[:, ts(h, 512)], lhsT=ones,
                             rhs=abst[:, ts(h, 512)], start=True, stop=True)
        ps_b = pspool.tile([P, dout], f32, tag="ps", name="ps_bias")
        for h in range(NB):
            nc.tensor.matmul(ps_b[:, ts(h, 512)], lhsT=ones,
                             rhs=abst[:, dout + h * 512:dout + (h + 1) * 512],
                             start=True, stop=True)
        alphaB = consts.tile([P, dout], f32)
        biasB = consts.tile([P, dout], f32)
        nc.vector.tensor_scalar_add(alphaB, ps_a, 0.0)
        nc.vector.tensor_scalar_add(biasB, ps_b, 0.0)

        def epilogue(psd, t):
            osb = opool.tile([P, dout], f32, tag="osb", name=f"osb{t}")
            nc.vector.tensor_mul(osb, psd, alphaB)
            nc.vector.tensor_add(osb, osb, biasB)
            nc.scalar.dma_start(out=out[ts(t, P), :], in_=osb)

        def mm_pair(psd, t, kp):
            for b2 in range(NB):
                nc.tensor.matmul(
                    psd[:, ts(b2, 512)],
                    lhsT=xbs[t][:, 2 * kp:2 * kp + 2, :],
                    rhs=wsb[:, 2 * kp:2 * kp + 2, ts(b2, 512)],
                    start=(kp == 0),
                    stop=(kp == KP - 1),
                    perf_mode=mybir.MatmulPerfMode.DoubleRow,
                )

        # --- leading tiles 0-3: k-pair-interleaved rounds -------------
        # near-zero lags: all four tiles consume W k-pair r in round r,
        # so PE demand per unit W delivered is 4x a single tile's, which
        # keeps consumption below the early DMA delivery rate; staggered
        # by one round so the four epilogues (and PSUM frees) stagger too
        LAGS = (0, 1, 2, 3)
        pstile = {}
        for t in range(4):
            pstile[t] = pspool.tile([P, dout], f32, tag="ps", name=f"ps_t{t}")
        for r in range(KP + LAGS[-1]):
            for t in range(4):
                kp = r - LAGS[t]
                if 0 <= kp < KP:
                    mm_pair(pstile[t], t, kp)
                    if kp == KP - 1:
                        epilogue(pstile[t], t)

        # --- steady-state tiles ---------------------------------------
        for t in range(4, TT):
            x_alloc(t)
            x_dma(nc.sync, t, 0, 32)
            if t < TT - 2:
                psd = pspool.tile([P, dout], f32, tag="ps", name=f"ps_t{t}")
                for kp in range(KP):
                    mm_pair(psd, t, kp)
                epilogue(psd, t)
            else:
                # tail tiles: chunked, each chunk with its OWN psum tile
                # (a chunk's start=True matmul WAR-depends on the whole
                # previous tile -> separate tiles avoid serializing
                # against the prior chunk's epilogue read).  The very
                # last chunk is narrow so the final epilogue + store
                # trail as little as possible behind the last matmul;
                # stores alternate the two HWDGE rings so posting and
                # completion overlap.
                widths = [512, 512] if t == TT - 2 else [320, 320, 256, 128]
                c0 = 0
                for q, w in enumerate(widths):
                    psq = pspool.tile([P, w], f32, tag="ps",
                                      name=f"ps_t{t}_q{q}")
                    for kp in range(KP):
                        nc.tensor.matmul(
                            psq,
                            lhsT=xbs[t][:, 2 * kp:2 * kp + 2, :],
                            rhs=wsb[:, 2 * kp:2 * kp + 2, c0:c0 + w],
                            start=(kp == 0),
                            stop=(kp == KP - 1),
                            perf_mode=mybir.MatmulPerfMode.DoubleRow,
                        )
                    oq = opool.tile([P, w], f32, tag="osb",
                                    name=f"osb_q{t}_{q}")
                    nc.vector.tensor_mul(oq, psq, alphaB[:, c0:c0 + w])
                    nc.vector.tensor_add(oq, oq, biasB[:, c0:c0 + w])
                    oeng = nc.sync if q % 2 == 0 else nc.scalar
                    oeng.dma_start(out=out[ts(t, P), c0:c0 + w], in_=oq)
                    c0 += w
    nc.finalize()
    return nc


def _split3_bf16(v):
    """Exact 3-way bf16 split: v == hi + mid + lo (f32 sum), elementwise."""
    import ml_dtypes

    bf16 = ml_dtypes.bfloat16
    v = np.asarray(v, dtype=np.float32)
    hi = v.astype(bf16)
    r = v - hi.astype(np.float32)
    mid = r.astype(bf16)
    r2 = r - mid.astype(np.float32)
    lo = r2.astype(bf16)
    assert (hi.astype(np.float32) + mid.astype(np.float32)
            + lo.astype(np.float32) == v).all(), "bf16 3-split not exact"
    return np.stack([hi, mid, lo])


def _shard_inputs(x, weight, bias, alpha):
    import ml_dtypes

    fp8 = ml_dtypes.float8_e4m3
    P = 128
    KT = DIN // P
    KP = KT // 2
    TT = TOK // P

    x2 = np.asarray(x, dtype=np.float32).reshape(NTOK, DIN)
    w = np.asarray(weight, dtype=np.float32)
    bias = np.asarray(bias, dtype=np.float32).reshape(-1)
    alpha_f = np.asarray(alpha, dtype=np.float32).reshape(-1)

    xTs = []
    for r in range(R):
        xT = np.sign(x2[r * TOK:(r + 1) * TOK, :].T).astype(fp8)  # [DIN, TOK]
        # -> [TT, 128(p), KT, 128(t)]: each [p, kt, t] token-tile contiguous
        xt_tiled = np.ascontiguousarray(
            xT.reshape(KT, P, TT, P).transpose(2, 1, 0, 3))
        xTs.append(xt_tiled)

    wT8 = np.sign(w.T).astype(fp8)  # [DIN, DOUT_FULL], values in {-1, 0, +1}
    wts, abs3 = [], []
    for c in range(C):
        blk = wT8[:, c * DOUT:(c + 1) * DOUT]  # [DIN, DOUT]
        # -> [KP, 128(p), 2, DOUT]: each k-pair contiguous, partition p
        # holds din rows 256*kp + p and 256*kp + 128 + p
        wt_tiled = np.ascontiguousarray(
            blk.reshape(KP, 2, P, DOUT).transpose(0, 2, 1, 3))
        wts.append(wt_tiled)
        ab = np.concatenate([alpha_f[c * DOUT:(c + 1) * DOUT],
                             bias[c * DOUT:(c + 1) * DOUT]])
        abs3.append(_split3_bf16(ab))

    in_maps = []
    for i in range(8):
        r, c = divmod(i, C)
        in_maps.append({"xt": xTs[r], "wt": wts[c], "ab3": abs3[c]})
    return in_maps


def kernel(x, weight, bias, alpha, _trace=False, _trace_cores=None):
    global LAST_RESULTS
    from concourse.bass_utils import run_bass_kernel_spmd

    in_maps = _shard_inputs(x, weight, bias, alpha)
    nc = build_nc()
    kwargs = {}
    if _trace:
        kwargs = dict(trace=True, trace_cores=_trace_cores or [0])
    res = run_bass_kernel_spmd(nc, in_maps, core_ids=list(range(8)), **kwargs)
    LAST_RESULTS = res

    out = np.empty((NTOK, DOUT_FULL), dtype=np.float32)
    for i in range(8):
        r, c = divmod(i, C)
        out[r * TOK:(r + 1) * TOK, c * DOUT:(c + 1) * DOUT] = res.results[i]["out"]
    return out.reshape(B, S, DOUT_FULL)
